# revision 13
# baseline (speedup 1.0000x reference)
"""Distributed attention kernel for Trainium2 (8 NeuronCores).

Reference computation (B=2, N=2048, C=1024, H=16, D=64, ALPHA=0.5):
    qkv = x @ W_qkv -> q,k,v [B,H,N,D]
    attn = softmax(q @ k^T / sqrt(D))
    attn = 0.5*dm + 0.5*attn
    out  = (attn @ v).reshape(B,N,C) @ W_proj + b_proj

Sharding: 8 cores = 2 batches x 4 head-groups (4 heads each).
Each core computes its head-group's slice end-to-end, including a partial
projection (row-slice of W_proj); host sums the 4 partials per batch.

Speed strategy vs the fp16 baseline: every matmul whose streams tolerate it
runs as an fp8e4m3 DoubleRow matmul (0.5 PE cycles per output column AND 2
contraction rows per partition), with hi+lo error compensation so accuracy
stays at ~fp16 level:
  - qkv projections: x and W split hi/lo fp8 (W pre-scaled x32 so W~N(0,1)
    quantizes in fp8 normal range); 3 accumulation terms hh+hl+lh at 0.75x
    the fp16 PE cost with 256-deep contraction per instruction.
  - dm@v: DoubleRow with slots = m-tile pairs; hi/lo on both dm (host,
    pre-scaled x512) and v (device, unscaled -- a 1/32 downscale would push
    the lo-residual under the fp8 subnormal floor): 3 terms at 0.75x.
  - proj: DoubleRow over the jo row-pair dim, hi/lo on outT and W_proj
    (both x32-scaled); the epilogue writes outT_hi/outT_lo fp8 directly.
  - scores and attn@v stay fp16: any single-fp8 quantization of the q/k or
    exp streams measures ~1.6-3e-2 on the 2e-2 gate (fp8's 3-bit mantissa
    puts ~3.6% rms noise on softmax weights), and hi/lo-compensating a
    moving operand costs exactly the DoubleRow speedup back.
  - exp: ScalarE, scale 2^-13 (undoes the 32x32 weight scales and applies
    D^-0.5) and bias -3.5 folded in; the bias cancels in softmax
    normalization (real q.k score tails reach 8.4 sigma on this input).
  - softmax denominator rides the e@v matmul as a 1/16 ones-column; the
    normalization constant 16/sum(e) lands the lambda=32 output scale that
    keeps outT in fp8-friendly range; pd's 512/32 and pout's 1/1024
    descales fold into existing copies.
"""

import numpy as np

B, N, C, H, D = 2, 2048, 1024, 16, 64
NCORES = 8
HG = 4                # head-groups per batch
HPC = H // HG         # heads per core = 4
DG = HPC * D          # 256: head-group width
SCALE = D ** -0.5

CT = 4                # contraction pair-tiles for qkv (1024 = 4*256)
NQ = N // 512         # 4 q-chunks
MT = N // 128         # 16 m (key) tiles
PT = MT // 2          # 8 m pair-tiles

WS = 32.0             # weight scale (W_qkv, W_proj)
LAM = 32.0            # output scale carried by outT
DMSC = 512.0          # dm host scale; pd = 512*dm@v, descaled 1/32 -> 16
ACT_SCALE = 0.125 / (WS * WS)   # 2^-13
EBIAS = -3.5
ONEC = 1.0 / 16.0     # denominator column: rec2 = 16/sum(e) = LAM*0.5/sum(e)


def _build_program():
    import concourse.bass as bass
    import concourse.bacc as bacc
    import concourse.tile as tile
    from concourse import mybir
    from contextlib import ExitStack

    f32 = mybir.dt.float32
    f16 = mybir.dt.float16
    f8 = mybir.dt.float8e4
    Exp = mybir.ActivationFunctionType.Exp
    DR = mybir.MatmulPerfMode.DoubleRow
    Alu = mybir.AluOpType

    nc = bacc.Bacc()
    xh = nc.declare_dram_parameter("xh", [128, CT, 2, N], f8, isOutput=False)
    xl = nc.declare_dram_parameter("xl", [128, CT, 2, N], f8, isOutput=False)
    wqh = nc.declare_dram_parameter("wqh", [128, CT, 2, DG], f8, isOutput=False)
    wql = nc.declare_dram_parameter("wql", [128, CT, 2, DG], f8, isOutput=False)
    wkh = nc.declare_dram_parameter("wkh", [128, CT, 2, DG], f8, isOutput=False)
    wkl = nc.declare_dram_parameter("wkl", [128, CT, 2, DG], f8, isOutput=False)
    wvh = nc.declare_dram_parameter("wvh", [128, CT, 2, DG], f8, isOutput=False)
    wvl = nc.declare_dram_parameter("wvl", [128, CT, 2, DG], f8, isOutput=False)
    wph = nc.declare_dram_parameter("wph", [128, 2, C], f8, isOutput=False)
    wpl = nc.declare_dram_parameter("wpl", [128, 2, C], f8, isOutput=False)
    dmh = nc.declare_dram_parameter("dmh", [128, PT, 2, N], f8, isOutput=False)
    dml = nc.declare_dram_parameter("dml", [128, PT, 2, N], f8, isOutput=False)
    pout = nc.declare_dram_parameter("pout", [C, N], f16, isOutput=True)

    with tile.TileContext(nc) as tc, ExitStack() as ctx:
        big = ctx.enter_context(tc.tile_pool(name="big", bufs=1))
        epool = ctx.enter_context(tc.tile_pool(name="epool", bufs=6))
        small = ctx.enter_context(tc.tile_pool(name="small", bufs=1))
        outp = ctx.enter_context(tc.tile_pool(name="outp", bufs=4))
        # PSUM: psS 2x[128,1024] (4 banks) + pe0/pe1 (2) + pd (1) + proj (1) = 8
        psS = ctx.enter_context(tc.tile_pool(name="psS", bufs=2, space="PSUM"))
        psE = ctx.enter_context(tc.tile_pool(name="psE", bufs=1, space="PSUM"))
        psD = ctx.enter_context(tc.tile_pool(name="psD", bufs=1, space="PSUM"))

        xh_s = big.tile([128, CT, 2, N], f8)
        xl_s = big.tile([128, CT, 2, N], f8)
        wq_s = {hl: big.tile([128, CT, 2, DG], f8, name=f"wq_s{hl}") for hl in range(2)}
        wk_s = {hl: big.tile([128, CT, 2, DG], f8, name=f"wk_s{hl}") for hl in range(2)}
        wv_s = {hl: big.tile([128, CT, 2, DG], f8, name=f"wv_s{hl}") for hl in range(2)}
        wp_s = {hl: big.tile([128, 2, C], f8, name=f"wp_s{hl}") for hl in range(2)}
        dm_s = {hl: big.tile([128, PT, 2, N], f8, name=f"dm_s{hl}") for hl in range(2)}
        qt = big.tile([128, 2, N], f16)
        kt = big.tile([128, 2, N], f16)
        # e@v stationary: [p, mt, head, D + ones-col], fp16, v unscaled
        vaug = big.tile([128, MT, HPC, D + 1], f16)
        # dm@v stationary: [p, (hi,lo), pt, m-parity, dg], fp8, v unscaled
        vd = big.tile([128, 2, PT, 2, DG], f8)
        oth = big.tile([128, 2, N], f8)
        otl = big.tile([128, 2, N], f8)
        bias_t = big.tile([128, 1], f32)
        ones16 = big.tile([1, D], f16)
        rscratch = nc.dram_tensor("rscratch", [8, 1024], f32)

        nc.vector.memset(bias_t[:, :], EBIAS)
        nc.vector.memset(ones16[:, :], 1.0)
        nc.vector.memset(vaug[:, :, :, D], ONEC)

        # ---- input DMAs: x in (ctpair x n-half) chunks so production can
        # start at ~1.6us and the first-half groups finish by ~7us ----
        for half in range(2):
            nsl = slice(half * 1024, (half + 1) * 1024)
            for cp in range(CT):
                nc.sync.dma_start(out=xh_s[:, cp, :, nsl], in_=xh[:, cp, :, nsl])
                nc.sync.dma_start(out=xl_s[:, cp, :, nsl], in_=xl[:, cp, :, nsl])
                if half == 0 and cp == 0:
                    nc.sync.dma_start(out=wk_s[0][:, :, :, :], in_=wkh[:, :, :, :])
                    nc.sync.dma_start(out=wk_s[1][:, :, :, :], in_=wkl[:, :, :, :])
                    nc.sync.dma_start(out=wq_s[0][:, :, :, :], in_=wqh[:, :, :, :])
                    nc.sync.dma_start(out=wq_s[1][:, :, :, :], in_=wql[:, :, :, :])
        nc.sync.dma_start(out=wv_s[0][:, :, :, :], in_=wvh[:, :, :, :])
        nc.sync.dma_start(out=wv_s[1][:, :, :, :], in_=wvl[:, :, :, :])
        nc.sync.dma_start(out=wp_s[0][:, :, :], in_=wph[:, :, :])
        nc.sync.dma_start(out=wp_s[1][:, :, :], in_=wpl[:, :, :])
        for pt in range(PT):
            nc.sync.dma_start(out=dm_s[0][:, pt, :, :], in_=dmh[:, pt, :, :])
            nc.sync.dma_start(out=dm_s[1][:, pt, :, :], in_=dml[:, pt, :, :])

        # 3 hi/lo accumulation terms: (x_hi,w_hi), (x_hi,w_lo), (x_lo,w_hi)
        def terms(w):
            return ((xh_s, w[0]), (xh_s, w[1]), (xl_s, w[0]))

        # ---- phase 1: q/k production, ct-outer over 6 psum slots per round
        # (round A = n-half 0 groups, round B = n-half 1), overlapping x DMA ----
        slot_tags = ["psS", "psS", "pe0", "pe1", "pd0", "pd1"]

        def qk_round(groups):
            ps = {}
            for i, (w, dst, jo, nq) in enumerate(groups):
                tag = slot_tags[i]
                pool = psS if tag == "psS" else (psE if tag.startswith("pe") else psD)
                ps[i] = pool.tile([128, 512], f32, name=f"g{i}", tag=tag)
            for cp in range(CT):
                for ti in range(3):
                    for i, (w, dst, jo, nq) in enumerate(groups):
                        xs, ws = terms(w)[ti]
                        nc.tensor.matmul(
                            ps[i][:, :],
                            lhsT=ws[:, cp, :, jo * 128:(jo + 1) * 128],
                            rhs=xs[:, cp, :, nq * 512:(nq + 1) * 512],
                            start=(cp == 0 and ti == 0),
                            stop=(cp == CT - 1 and ti == 2),
                            perf_mode=DR,
                        )
            order = sorted(range(len(groups)),
                           key=lambda i: 0 if slot_tags[i] in ("pe0", "pe1") else 1)
            for i in order:
                w, dst, jo, nq = groups[i]
                nc.vector.tensor_copy(dst[:, jo, nq * 512:(nq + 1) * 512], ps[i][:, :])

        qk_round([(wk_s, kt, 0, 0), (wk_s, kt, 0, 1), (wk_s, kt, 1, 0),
                  (wk_s, kt, 1, 1), (wq_s, qt, 0, 0), (wq_s, qt, 0, 1)])

        # ---- v tiles: DR production; epilogue emits vaug fp16 + vd hi/lo fp8 ----
        def v_tile(mt):
            msl = slice(mt * 128, (mt + 1) * 128)
            ps = psE.tile([128, DG], f32, name="ps", tag=f"pe{mt % 2}",
                          padded_shape=[128, 512])
            for cp in range(CT):
                for ti, (xs, ws) in enumerate(terms(wv_s)):
                    nc.tensor.matmul(
                        ps[:, :],
                        lhsT=xs[:, cp, :, msl],
                        rhs=ws[:, cp, :, :],
                        start=(cp == 0 and ti == 0), stop=(cp == CT - 1 and ti == 2),
                        perf_mode=DR,
                    )
            psv = ps[:, :]
            ps4 = bass.AP(tensor=psv.tensor, offset=psv.offset,
                          ap=[list(psv.ap[0]), [D, HPC], [1, D]])
            pt, par = divmod(mt, 2)
            nc.vector.tensor_scalar_mul(vaug[:, mt, :, 0:D], ps4, 1.0 / WS)
            hi = vd[:, 0, pt, par, :]
            nc.vector.tensor_scalar_mul(hi, psv, 1.0 / WS)
            nc.vector.scalar_tensor_tensor(
                vd[:, 1, pt, par, :], psv, 1.0 / WS, hi,
                op0=Alu.mult, op1=Alu.subtract,
            )

        for mt in range(8):
            v_tile(mt)
        qk_round([(wk_s, kt, 0, 2), (wk_s, kt, 0, 3), (wk_s, kt, 1, 2),
                  (wk_s, kt, 1, 3), (wq_s, qt, 0, 2), (wq_s, qt, 0, 3)])
        for mt in range(8, MT):
            v_tile(mt)
        qk_round([(wq_s, qt, 1, 0), (wq_s, qt, 1, 1),
                  (wq_s, qt, 1, 2), (wq_s, qt, 1, 3)])

        # ---- phase 2: attention ----
        def proj_group(nq, co, tag="pd1"):
            qsl = slice(nq * 512, (nq + 1) * 512)
            if tag == "pd1":
                ps = psD.tile([128, 512], f32, name="pj", tag="pd1")
            else:
                ps = psS.tile([128, 512], f32, name="pjS", tag="psS",
                              padded_shape=[128, 1024])
            for ti, (o_s, w_s) in enumerate(((oth, wp_s[0]), (otl, wp_s[0]), (oth, wp_s[1]))):
                nc.tensor.matmul(
                    ps[:, :],
                    lhsT=w_s[:, :, co * 128:(co + 1) * 128],
                    rhs=o_s[:, :, qsl],
                    start=(ti == 0), stop=(ti == 2),
                    perf_mode=DR,
                )
            so = outp.tile([128, 512], f16)
            nc.vector.tensor_scalar_mul(so[:, :], ps[:, :], 1.0 / (WS * LAM))
            nc.sync.dma_start(out=pout[co * 128:(co + 1) * 128, qsl], in_=so[:, :])

        pending_proj = None
        for nq in range(NQ):
            qsl = slice(nq * 512, (nq + 1) * 512)
            for hp in range(2):
                pe0 = psE.tile([D + 1, 512], f32, name="pe0", tag="pe0",
                               padded_shape=[128, 512])
                pe1 = psE.tile([D + 1, 512], f32, name="pe1", tag="pe1",
                               padded_shape=[128, 512])
                pd = psD.tile([128, 512], f32, name="pd", tag="pd0")
                pes = (pe0, pe1)
                for mt in range(MT):
                    msl = slice(mt * 128, (mt + 1) * 128)
                    if mt % 2 == 0:
                        pt = mt // 2
                        # dm@v: slots = m-tile pairs; 3 hi/lo terms
                        for ti, (vhl, dhl) in enumerate(((0, 0), (1, 0), (0, 1))):
                            nc.tensor.matmul(
                                pd[:, :],
                                lhsT=vd[:, vhl, pt, :, hp * 128:(hp + 1) * 128],
                                rhs=dm_s[dhl][:, pt, :, qsl],
                                start=(pt == 0 and ti == 0),
                                stop=(pt == PT - 1 and ti == 2),
                                perf_mode=DR,
                            )
                    sps = psS.tile([128, 1024], f32, name="sps", tag="psS")
                    nc.tensor.matmul(
                        sps[:, 0:512],
                        lhsT=kt[0:D, hp, msl], rhs=qt[0:D, hp, qsl],
                        start=True, stop=True,
                    )
                    nc.tensor.matmul(
                        sps[:, 512:1024],
                        lhsT=kt[D:2 * D, hp, msl], rhs=qt[D:2 * D, hp, qsl],
                        start=True, stop=True,
                    )
                    et = epool.tile([128, 1024], f16)
                    nc.scalar.activation(et[:, :], sps[:, :], Exp,
                                         bias=bias_t[:, :], scale=ACT_SCALE)
                    nc.tensor.matmul(
                        pe0[:, :], lhsT=vaug[:, mt, 2 * hp, :], rhs=et[:, 0:512],
                        start=(mt == 0), stop=(mt == MT - 1),
                    )
                    nc.tensor.matmul(
                        pe1[:, :], lhsT=vaug[:, mt, 2 * hp + 1, :], rhs=et[:, 512:1024],
                        start=(mt == 0), stop=(mt == MT - 1),
                    )
                    if pending_proj is not None and hp == 0 and 1 <= mt <= 8:
                        proj_group(pending_proj, mt - 1)
                # epilogue: normalize softmax part, add dm part, emit outT hi/lo
                slot = nq * 2 + hp
                last = (nq == NQ - 1 and hp == 1)
                if last:
                    pe_s0, pe_s1, pd_s = pe0, pe1, None
                else:
                    pe_s0 = small.tile([D + 1, 512], f32, name="pe_s0", tag="pe_s0")
                    nc.vector.tensor_copy(pe_s0[:, :], pe0[:, :])
                    pe_s1 = small.tile([D + 1, 512], f32, name="pe_s1", tag="pe_s1")
                    nc.vector.tensor_copy(pe_s1[:, :], pe1[:, :])
                    pd_s = small.tile([128, 512], f32, name="pd_s", tag="pd_s")
                    nc.vector.tensor_scalar_mul(pd_s[:, :], pd[:, :], 1.0 / LAM)
                rec2 = small.tile([1, 1024], f16 if last else f32, name="rec2",
                                  tag="rec2l" if last else "rec2")
                for half, ps_ in ((0, pe_s0), (1, pe_s1)):
                    with nc.allow_low_precision(reason="1/r broadcast"):
                        nc.vector.reciprocal(
                            rec2[:, half * 512:(half + 1) * 512], ps_[D:D + 1, :])
                if last:
                    bcp = psS.tile([D, 1024], f32, name="bcp", tag="psS",
                                   padded_shape=[128, 1024])
                    nc.tensor.matmul(bcp[:, 0:512], lhsT=ones16[:, :],
                                     rhs=rec2[:, 0:512], start=True, stop=True)
                    nc.tensor.matmul(bcp[:, 512:1024], lhsT=ones16[:, :],
                                     rhs=rec2[:, 512:1024], start=True, stop=True)
                    bcs = small.tile([D, 1024], f32, name="bcs", tag="bcs")
                    nc.vector.tensor_copy(bcs[:, :], bcp[:, :])
                else:
                    nc.sync.dma_start(out=rscratch[slot:slot + 1, :], in_=rec2[:, :])
                    row = rscratch[slot, :]
                    bc_ap = bass.AP(tensor=row.tensor, offset=row.offset,
                                    ap=[[0, D]] + list(row.ap))
                    bcs = small.tile([D, 1024], f32, name="bcs", tag="bcs")
                    nc.sync.dma_start(out=bcs[:, :], in_=bc_ap)
                for half, ps_ in ((0, pe_s0), (1, pe_s1)):
                    hsl = slice(half * D, (half + 1) * D)
                    t1 = small.tile([128, 512], f32, name="t1", tag="t1")
                    nc.vector.tensor_mul(
                        t1[hsl, :], ps_[0:D, :], bcs[:, half * 512:(half + 1) * 512])
                    t2 = small.tile([128, 512], f16, name="t2", tag="t2")
                    if last:
                        nc.vector.scalar_tensor_tensor(
                            t2[hsl, :], pd[hsl, :], 1.0 / LAM, t1[hsl, :],
                            op0=Alu.mult, op1=Alu.add,
                        )
                    else:
                        nc.vector.tensor_add(t2[hsl, :], t1[hsl, :], pd_s[hsl, :])
                    hi = oth[hsl, hp, qsl]
                    nc.vector.tensor_copy(hi, t2[hsl, :])
                    nc.vector.scalar_tensor_tensor(
                        otl[hsl, hp, qsl], t2[hsl, :], 1.0, hi,
                        op0=Alu.mult, op1=Alu.subtract,
                    )
            pending_proj = nq
        for co in range(C // 128):
            proj_group(NQ - 1, co, tag="pd1" if co % 2 == 0 else "psS")
    nc.compile()
    return nc


_PROGRAM = None


def _get_program():
    global _PROGRAM
    if _PROGRAM is None:
        _PROGRAM = _build_program()
    return _PROGRAM


def _hilo(a, f8):
    hi = np.asarray(a, dtype=f8)
    lo = np.asarray(a - hi.astype(np.float32), dtype=f8)
    return hi, lo


def _pairct(a, nt):
    """[K, F] -> [128, nt, 2, F] with (p, t, i) <-> row t*256 + i*128 + p."""
    K, F = a.shape
    assert K == nt * 256
    return np.ascontiguousarray(a.reshape(nt, 2, 128, F).transpose(2, 0, 1, 3))


def _make_in_maps(x, distance_matrix, W_qkv, W_proj):
    import ml_dtypes
    f8 = ml_dtypes.float8_e4m3

    in_maps = []
    xTs = [np.ascontiguousarray(x[b].T).astype(np.float32) for b in range(B)]
    dmSs = [np.ascontiguousarray(DMSC * distance_matrix[b, 0].T).astype(np.float32)
            for b in range(B)]
    dm_pairs = []
    for b in range(B):
        dh, dl = _hilo(dmSs[b], f8)
        dm_pairs.append((_pairct(dh, PT), _pairct(dl, PT)))
    x_pairs = []
    for b in range(B):
        xhi, xlo = _hilo(xTs[b], f8)
        x_pairs.append((_pairct(xhi, CT), _pairct(xlo, CT)))

    for core in range(NCORES):
        b, hg = divmod(core, HG)
        sl = slice(hg * DG, (hg + 1) * DG)
        wq = WS * W_qkv[:, sl].astype(np.float32)
        wk = WS * W_qkv[:, C + hg * DG:C + (hg + 1) * DG].astype(np.float32)
        wv = WS * W_qkv[:, 2 * C + hg * DG:2 * C + (hg + 1) * DG].astype(np.float32)
        wp = WS * W_proj[sl, :].astype(np.float32)
        wqh_, wql_ = _hilo(wq, f8)
        wkh_, wkl_ = _hilo(wk, f8)
        wvh_, wvl_ = _hilo(wv, f8)
        wph_, wpl_ = _hilo(wp, f8)
        in_maps.append({
            "xh": x_pairs[b][0], "xl": x_pairs[b][1],
            "wqh": _pairct(wqh_, CT), "wql": _pairct(wql_, CT),
            "wkh": _pairct(wkh_, CT), "wkl": _pairct(wkl_, CT),
            "wvh": _pairct(wvh_, CT), "wvl": _pairct(wvl_, CT),
            "wph": np.ascontiguousarray(wph_.reshape(2, 128, C).transpose(1, 0, 2)),
            "wpl": np.ascontiguousarray(wpl_.reshape(2, 128, C).transpose(1, 0, 2)),
            "dmh": dm_pairs[b][0], "dml": dm_pairs[b][1],
        })
    return in_maps


def kernel(x, distance_matrix, W_qkv, W_proj, b_proj, _results_hook=None):
    from concourse.bass_utils import run_bass_kernel_spmd

    x = np.asarray(x)
    distance_matrix = np.asarray(distance_matrix)
    W_qkv = np.asarray(W_qkv)
    W_proj = np.asarray(W_proj)
    b_proj = np.asarray(b_proj)
    nc = _get_program()
    in_maps = _make_in_maps(x, distance_matrix, W_qkv, W_proj)
    res = run_bass_kernel_spmd(nc, in_maps, list(range(NCORES)))
    if _results_hook is not None:
        _results_hook(res)
    out = np.zeros((B, N, C), dtype=np.float32)
    for core in range(NCORES):
        b = core // HG
        out[b] += res.results[core]["pout"].T.astype(np.float32)
    out += b_proj[None, None, :].astype(np.float32)
    return out


# revision 14
# speedup vs baseline: 1.0050x; 1.0050x over previous
"""Distributed attention kernel for Trainium2 (8 NeuronCores).

Reference computation (B=2, N=2048, C=1024, H=16, D=64, ALPHA=0.5):
    qkv = x @ W_qkv -> q,k,v [B,H,N,D]
    attn = softmax(q @ k^T / sqrt(D))
    attn = 0.5*dm + 0.5*attn
    out  = (attn @ v).reshape(B,N,C) @ W_proj + b_proj

Sharding: 8 cores = 2 batches x 4 head-groups (4 heads each).
Each core computes its head-group's slice end-to-end, including a partial
projection (row-slice of W_proj); host sums the 4 partials per batch.

Speed strategy vs the fp16 baseline: every matmul whose streams tolerate it
runs as an fp8e4m3 DoubleRow matmul (0.5 PE cycles per output column AND 2
contraction rows per partition), with hi+lo error compensation so accuracy
stays at ~fp16 level:
  - qkv projections: x and W split hi/lo fp8 (W pre-scaled x32 so W~N(0,1)
    quantizes in fp8 normal range); 3 accumulation terms hh+hl+lh at 0.75x
    the fp16 PE cost with 256-deep contraction per instruction.
  - dm@v: DoubleRow with slots = m-tile pairs; hi/lo on both dm (host,
    pre-scaled x512) and v (device, unscaled -- a 1/32 downscale would push
    the lo-residual under the fp8 subnormal floor): 3 terms at 0.75x.
  - proj: DoubleRow over the jo row-pair dim, hi/lo on outT and W_proj
    (both x32-scaled); the epilogue writes outT_hi/outT_lo fp8 directly.
  - scores and attn@v stay fp16: any single-fp8 quantization of the q/k or
    exp streams measures ~1.6-3e-2 on the 2e-2 gate (fp8's 3-bit mantissa
    puts ~3.6% rms noise on softmax weights), and hi/lo-compensating a
    moving operand costs exactly the DoubleRow speedup back.
  - exp: ScalarE, scale 2^-13 (undoes the 32x32 weight scales and applies
    D^-0.5) and bias -3.5 folded in; the bias cancels in softmax
    normalization (real q.k score tails reach 8.4 sigma on this input).
  - softmax denominator rides the e@v matmul as a 1/16 ones-column; the
    normalization constant 16/sum(e) lands the lambda=32 output scale that
    keeps outT in fp8-friendly range; pd's 512/32 and pout's 1/1024
    descales fold into existing copies.
"""

import numpy as np

B, N, C, H, D = 2, 2048, 1024, 16, 64
NCORES = 8
HG = 4                # head-groups per batch
HPC = H // HG         # heads per core = 4
DG = HPC * D          # 256: head-group width
SCALE = D ** -0.5

CT = 4                # contraction pair-tiles for qkv (1024 = 4*256)
NQ = N // 512         # 4 q-chunks
MT = N // 128         # 16 m (key) tiles
PT = MT // 2          # 8 m pair-tiles

WS = 32.0             # weight scale (W_qkv, W_proj)
LAM = 32.0            # output scale carried by outT
DMSC = 512.0          # dm host scale; pd = 512*dm@v, descaled 1/32 -> 16
ACT_SCALE = 0.125 / (WS * WS)   # 2^-13
EBIAS = -3.5
ONEC = 1.0 / 16.0     # denominator column: rec2 = 16/sum(e) = LAM*0.5/sum(e)


def _build_program():
    import concourse.bass as bass
    import concourse.bacc as bacc
    import concourse.tile as tile
    from concourse import mybir
    from contextlib import ExitStack

    f32 = mybir.dt.float32
    f16 = mybir.dt.float16
    f8 = mybir.dt.float8e4
    Exp = mybir.ActivationFunctionType.Exp
    DR = mybir.MatmulPerfMode.DoubleRow
    Alu = mybir.AluOpType

    nc = bacc.Bacc()
    xh = nc.declare_dram_parameter("xh", [128, CT, 2, N], f8, isOutput=False)
    xl = nc.declare_dram_parameter("xl", [128, CT, 2, N], f8, isOutput=False)
    wqh = nc.declare_dram_parameter("wqh", [128, CT, 2, DG], f8, isOutput=False)
    wql = nc.declare_dram_parameter("wql", [128, CT, 2, DG], f8, isOutput=False)
    wkh = nc.declare_dram_parameter("wkh", [128, CT, 2, DG], f8, isOutput=False)
    wkl = nc.declare_dram_parameter("wkl", [128, CT, 2, DG], f8, isOutput=False)
    wvh = nc.declare_dram_parameter("wvh", [128, CT, 2, DG], f8, isOutput=False)
    wvl = nc.declare_dram_parameter("wvl", [128, CT, 2, DG], f8, isOutput=False)
    wph = nc.declare_dram_parameter("wph", [128, 2, C], f8, isOutput=False)
    wpl = nc.declare_dram_parameter("wpl", [128, 2, C], f8, isOutput=False)
    dmh = nc.declare_dram_parameter("dmh", [128, PT, 2, N], f8, isOutput=False)
    dml = nc.declare_dram_parameter("dml", [128, PT, 2, N], f8, isOutput=False)
    pout = nc.declare_dram_parameter("pout", [C, N], f16, isOutput=True)

    with tile.TileContext(nc) as tc, ExitStack() as ctx:
        big = ctx.enter_context(tc.tile_pool(name="big", bufs=1))
        epool = ctx.enter_context(tc.tile_pool(name="epool", bufs=6))
        small = ctx.enter_context(tc.tile_pool(name="small", bufs=1))
        outp = ctx.enter_context(tc.tile_pool(name="outp", bufs=4))
        # PSUM: psS 2x[128,1024] (4 banks) + pe0/pe1 (2) + pd (1) + proj (1) = 8
        psS = ctx.enter_context(tc.tile_pool(name="psS", bufs=2, space="PSUM"))
        psE = ctx.enter_context(tc.tile_pool(name="psE", bufs=1, space="PSUM"))
        psD = ctx.enter_context(tc.tile_pool(name="psD", bufs=1, space="PSUM"))

        xh_s = big.tile([128, CT, 2, N], f8)
        xl_s = big.tile([128, CT, 2, N], f8)
        wq_s = {hl: big.tile([128, CT, 2, DG], f8, name=f"wq_s{hl}") for hl in range(2)}
        wk_s = {hl: big.tile([128, CT, 2, DG], f8, name=f"wk_s{hl}") for hl in range(2)}
        wv_s = {hl: big.tile([128, CT, 2, DG], f8, name=f"wv_s{hl}") for hl in range(2)}
        wp_s = {hl: big.tile([128, 2, C], f8, name=f"wp_s{hl}") for hl in range(2)}
        dm_s = {hl: big.tile([128, PT, 2, N], f8, name=f"dm_s{hl}") for hl in range(2)}
        qt = big.tile([128, 2, N], f16)
        kt = big.tile([128, 2, N], f16)
        # e@v stationary: [p, mt, head, D + ones-col], fp16, v unscaled
        vaug = big.tile([128, MT, HPC, D + 1], f16)
        # dm@v stationary: [p, (hi,lo), pt, m-parity, dg], fp8, v unscaled
        vd = big.tile([128, 2, PT, 2, DG], f8)
        oth = big.tile([128, 2, N], f8)
        otl = big.tile([128, 2, N], f8)
        bias_t = big.tile([128, 1], f32)
        ones16 = big.tile([1, D], f16)
        rscratch = nc.dram_tensor("rscratch", [8, 1024], f32)

        nc.vector.memset(bias_t[:, :], EBIAS)
        nc.vector.memset(ones16[:, :], 1.0)
        nc.vector.memset(vaug[:, :, :, D], ONEC)

        # ---- input DMAs: x in (ctpair x n-half) chunks so production can
        # start at ~1.6us and the first-half groups finish by ~7us ----
        for half in range(2):
            nsl = slice(half * 1024, (half + 1) * 1024)
            for cp in range(CT):
                nc.sync.dma_start(out=xh_s[:, cp, :, nsl], in_=xh[:, cp, :, nsl])
                nc.sync.dma_start(out=xl_s[:, cp, :, nsl], in_=xl[:, cp, :, nsl])
                if half == 0 and cp == 0:
                    nc.sync.dma_start(out=wk_s[0][:, :, :, :], in_=wkh[:, :, :, :])
                    nc.sync.dma_start(out=wk_s[1][:, :, :, :], in_=wkl[:, :, :, :])
                    nc.sync.dma_start(out=wq_s[0][:, :, :, :], in_=wqh[:, :, :, :])
                    nc.sync.dma_start(out=wq_s[1][:, :, :, :], in_=wql[:, :, :, :])
        nc.sync.dma_start(out=wv_s[0][:, :, :, :], in_=wvh[:, :, :, :])
        nc.sync.dma_start(out=wv_s[1][:, :, :, :], in_=wvl[:, :, :, :])
        nc.sync.dma_start(out=wp_s[0][:, :, :], in_=wph[:, :, :])
        nc.sync.dma_start(out=wp_s[1][:, :, :], in_=wpl[:, :, :])
        for pt in range(PT):
            nc.sync.dma_start(out=dm_s[0][:, pt, :, :], in_=dmh[:, pt, :, :])
            nc.sync.dma_start(out=dm_s[1][:, pt, :, :], in_=dml[:, pt, :, :])

        # 3 hi/lo accumulation terms: (x_hi,w_hi), (x_hi,w_lo), (x_lo,w_hi)
        def terms(w):
            return ((xh_s, w[0]), (xh_s, w[1]), (xl_s, w[0]))

        # ---- phase 1: q/k production, ct-outer over 6 psum slots per round
        # (round A = n-half 0 groups, round B = n-half 1), overlapping x DMA ----
        slot_tags = ["psS", "psS", "pe0", "pe1", "pd0", "pd1"]

        def qk_round(groups):
            ps = {}
            for i, (w, dst, jo, nq) in enumerate(groups):
                tag = slot_tags[i]
                pool = psS if tag == "psS" else (psE if tag.startswith("pe") else psD)
                ps[i] = pool.tile([128, 512], f32, name=f"g{i}", tag=tag)
            for cp in range(CT):
                for ti in range(3):
                    for i, (w, dst, jo, nq) in enumerate(groups):
                        xs, ws = terms(w)[ti]
                        nc.tensor.matmul(
                            ps[i][:, :],
                            lhsT=ws[:, cp, :, jo * 128:(jo + 1) * 128],
                            rhs=xs[:, cp, :, nq * 512:(nq + 1) * 512],
                            start=(cp == 0 and ti == 0),
                            stop=(cp == CT - 1 and ti == 2),
                            perf_mode=DR,
                        )
            order = sorted(range(len(groups)),
                           key=lambda i: 0 if slot_tags[i] in ("pe0", "pe1") else 1)
            for i in order:
                w, dst, jo, nq = groups[i]
                nc.vector.tensor_copy(dst[:, jo, nq * 512:(nq + 1) * 512], ps[i][:, :])

        qk_round([(wk_s, kt, 0, 0), (wk_s, kt, 0, 1), (wk_s, kt, 1, 0),
                  (wk_s, kt, 1, 1), (wq_s, qt, 0, 0), (wq_s, qt, 0, 1)])

        # ---- v tiles: DR production; epilogue emits vaug fp16 + vd hi/lo fp8 ----
        def v_tile(mt):
            msl = slice(mt * 128, (mt + 1) * 128)
            ps = psE.tile([128, DG], f32, name="ps", tag=f"pe{mt % 2}",
                          padded_shape=[128, 512])
            for cp in range(CT):
                for ti, (xs, ws) in enumerate(terms(wv_s)):
                    nc.tensor.matmul(
                        ps[:, :],
                        lhsT=xs[:, cp, :, msl],
                        rhs=ws[:, cp, :, :],
                        start=(cp == 0 and ti == 0), stop=(cp == CT - 1 and ti == 2),
                        perf_mode=DR,
                    )
            psv = ps[:, :]
            ps4 = bass.AP(tensor=psv.tensor, offset=psv.offset,
                          ap=[list(psv.ap[0]), [D, HPC], [1, D]])
            pt, par = divmod(mt, 2)
            nc.vector.tensor_scalar_mul(vaug[:, mt, :, 0:D], ps4, 1.0 / WS)
            hi = vd[:, 0, pt, par, :]
            nc.vector.tensor_scalar_mul(hi, psv, 1.0 / WS)
            nc.vector.scalar_tensor_tensor(
                vd[:, 1, pt, par, :], psv, 1.0 / WS, hi,
                op0=Alu.mult, op1=Alu.subtract,
            )

        for mt in range(MT):
            v_tile(mt)
        qk_round([(wk_s, kt, 0, 2), (wk_s, kt, 0, 3), (wk_s, kt, 1, 2),
                  (wk_s, kt, 1, 3), (wq_s, qt, 0, 2), (wq_s, qt, 0, 3)])
        qk_round([(wq_s, qt, 1, 0), (wq_s, qt, 1, 1),
                  (wq_s, qt, 1, 2), (wq_s, qt, 1, 3)])

        # ---- phase 2: attention ----
        def proj_group(nq, co, tag="pd1"):
            qsl = slice(nq * 512, (nq + 1) * 512)
            if tag == "pd1":
                ps = psD.tile([128, 512], f32, name="pj", tag="pd1")
            else:
                ps = psS.tile([128, 512], f32, name="pjS", tag="psS",
                              padded_shape=[128, 1024])
            for ti, (o_s, w_s) in enumerate(((oth, wp_s[0]), (otl, wp_s[0]), (oth, wp_s[1]))):
                nc.tensor.matmul(
                    ps[:, :],
                    lhsT=w_s[:, :, co * 128:(co + 1) * 128],
                    rhs=o_s[:, :, qsl],
                    start=(ti == 0), stop=(ti == 2),
                    perf_mode=DR,
                )
            so = outp.tile([128, 512], f16)
            nc.vector.tensor_scalar_mul(so[:, :], ps[:, :], 1.0 / (WS * LAM))
            nc.sync.dma_start(out=pout[co * 128:(co + 1) * 128, qsl], in_=so[:, :])

        pending_proj = None
        for nq in range(NQ):
            qsl = slice(nq * 512, (nq + 1) * 512)
            for hp in range(2):
                pe0 = psE.tile([D + 1, 512], f32, name="pe0", tag="pe0",
                               padded_shape=[128, 512])
                pe1 = psE.tile([D + 1, 512], f32, name="pe1", tag="pe1",
                               padded_shape=[128, 512])
                pd = psD.tile([128, 512], f32, name="pd", tag="pd0")
                pes = (pe0, pe1)
                for mt in range(MT):
                    msl = slice(mt * 128, (mt + 1) * 128)
                    sps = psS.tile([128, 1024], f32, name="sps", tag="psS")
                    nc.tensor.matmul(
                        sps[:, 0:512],
                        lhsT=kt[0:D, hp, msl], rhs=qt[0:D, hp, qsl],
                        start=True, stop=True,
                    )
                    nc.tensor.matmul(
                        sps[:, 512:1024],
                        lhsT=kt[D:2 * D, hp, msl], rhs=qt[D:2 * D, hp, qsl],
                        start=True, stop=True,
                    )
                    et = epool.tile([128, 1024], f16)
                    nc.scalar.activation(et[:, :], sps[:, :], Exp,
                                         bias=bias_t[:, :], scale=ACT_SCALE)
                    if mt % 2 == 0:
                        pt = mt // 2
                        # dm@v: slots = m-tile pairs; 3 hi/lo terms
                        for ti, (vhl, dhl) in enumerate(((0, 0), (1, 0), (0, 1))):
                            nc.tensor.matmul(
                                pd[:, :],
                                lhsT=vd[:, vhl, pt, :, hp * 128:(hp + 1) * 128],
                                rhs=dm_s[dhl][:, pt, :, qsl],
                                start=(pt == 0 and ti == 0),
                                stop=(pt == PT - 1 and ti == 2),
                                perf_mode=DR,
                            )
                    nc.tensor.matmul(
                        pe0[:, :], lhsT=vaug[:, mt, 2 * hp, :], rhs=et[:, 0:512],
                        start=(mt == 0), stop=(mt == MT - 1),
                    )
                    nc.tensor.matmul(
                        pe1[:, :], lhsT=vaug[:, mt, 2 * hp + 1, :], rhs=et[:, 512:1024],
                        start=(mt == 0), stop=(mt == MT - 1),
                    )
                    if pending_proj is not None and hp == 0 and 1 <= mt <= 8:
                        proj_group(pending_proj, mt - 1)
                # epilogue: normalize softmax part, add dm part, emit outT hi/lo
                slot = nq * 2 + hp
                last = (nq == NQ - 1 and hp == 1)
                if last:
                    pe_s0, pe_s1, pd_s = pe0, pe1, None
                else:
                    pd_s = small.tile([128, 512], f32, name="pd_s", tag="pd_s")
                    nc.vector.tensor_scalar_mul(pd_s[:, :], pd[:, :], 1.0 / LAM)
                    pe_s0 = small.tile([D + 1, 512], f32, name="pe_s0", tag="pe_s0")
                    nc.vector.tensor_copy(pe_s0[:, :], pe0[:, :])
                    pe_s1 = small.tile([D + 1, 512], f32, name="pe_s1", tag="pe_s1")
                    nc.vector.tensor_copy(pe_s1[:, :], pe1[:, :])
                rec2 = small.tile([1, 1024], f16 if last else f32, name="rec2",
                                  tag="rec2l" if last else "rec2")
                for half, ps_ in ((0, pe_s0), (1, pe_s1)):
                    with nc.allow_low_precision(reason="1/r broadcast"):
                        nc.vector.reciprocal(
                            rec2[:, half * 512:(half + 1) * 512], ps_[D:D + 1, :])
                if last:
                    bcp = psS.tile([D, 1024], f32, name="bcp", tag="psS",
                                   padded_shape=[128, 1024])
                    nc.tensor.matmul(bcp[:, 0:512], lhsT=ones16[:, :],
                                     rhs=rec2[:, 0:512], start=True, stop=True)
                    nc.tensor.matmul(bcp[:, 512:1024], lhsT=ones16[:, :],
                                     rhs=rec2[:, 512:1024], start=True, stop=True)
                    bcs = small.tile([D, 1024], f32, name="bcs", tag="bcs")
                    nc.vector.tensor_copy(bcs[:, :], bcp[:, :])
                else:
                    nc.sync.dma_start(out=rscratch[slot:slot + 1, :], in_=rec2[:, :])
                    row = rscratch[slot, :]
                    bc_ap = bass.AP(tensor=row.tensor, offset=row.offset,
                                    ap=[[0, D]] + list(row.ap))
                    bcs = small.tile([D, 1024], f32, name="bcs", tag="bcs")
                    nc.sync.dma_start(out=bcs[:, :], in_=bc_ap)
                for half, ps_ in ((0, pe_s0), (1, pe_s1)):
                    hsl = slice(half * D, (half + 1) * D)
                    t1 = small.tile([128, 512], f32, name="t1", tag="t1")
                    nc.vector.tensor_mul(
                        t1[hsl, :], ps_[0:D, :], bcs[:, half * 512:(half + 1) * 512])
                    t2 = small.tile([128, 512], f16, name="t2", tag="t2")
                    if last:
                        nc.vector.scalar_tensor_tensor(
                            t2[hsl, :], pd[hsl, :], 1.0 / LAM, t1[hsl, :],
                            op0=Alu.mult, op1=Alu.add,
                        )
                    else:
                        nc.vector.tensor_add(t2[hsl, :], t1[hsl, :], pd_s[hsl, :])
                    hi = oth[hsl, hp, qsl]
                    nc.vector.tensor_copy(hi, t2[hsl, :])
                    nc.vector.scalar_tensor_tensor(
                        otl[hsl, hp, qsl], t2[hsl, :], 1.0, hi,
                        op0=Alu.mult, op1=Alu.subtract,
                    )
            pending_proj = nq
        for co in range(C // 128):
            proj_group(NQ - 1, co, tag="pd1" if co % 2 == 0 else "psS")
    nc.compile()
    return nc


_PROGRAM = None


def _get_program():
    global _PROGRAM
    if _PROGRAM is None:
        _PROGRAM = _build_program()
    return _PROGRAM


def _hilo(a, f8):
    hi = np.asarray(a, dtype=f8)
    lo = np.asarray(a - hi.astype(np.float32), dtype=f8)
    return hi, lo


def _pairct(a, nt):
    """[K, F] -> [128, nt, 2, F] with (p, t, i) <-> row t*256 + i*128 + p."""
    K, F = a.shape
    assert K == nt * 256
    return np.ascontiguousarray(a.reshape(nt, 2, 128, F).transpose(2, 0, 1, 3))


def _make_in_maps(x, distance_matrix, W_qkv, W_proj):
    import ml_dtypes
    f8 = ml_dtypes.float8_e4m3

    in_maps = []
    xTs = [np.ascontiguousarray(x[b].T).astype(np.float32) for b in range(B)]
    dmSs = [np.ascontiguousarray(DMSC * distance_matrix[b, 0].T).astype(np.float32)
            for b in range(B)]
    dm_pairs = []
    for b in range(B):
        dh, dl = _hilo(dmSs[b], f8)
        dm_pairs.append((_pairct(dh, PT), _pairct(dl, PT)))
    x_pairs = []
    for b in range(B):
        xhi, xlo = _hilo(xTs[b], f8)
        x_pairs.append((_pairct(xhi, CT), _pairct(xlo, CT)))

    for core in range(NCORES):
        b, hg = divmod(core, HG)
        sl = slice(hg * DG, (hg + 1) * DG)
        wq = WS * W_qkv[:, sl].astype(np.float32)
        wk = WS * W_qkv[:, C + hg * DG:C + (hg + 1) * DG].astype(np.float32)
        wv = WS * W_qkv[:, 2 * C + hg * DG:2 * C + (hg + 1) * DG].astype(np.float32)
        wp = WS * W_proj[sl, :].astype(np.float32)
        wqh_, wql_ = _hilo(wq, f8)
        wkh_, wkl_ = _hilo(wk, f8)
        wvh_, wvl_ = _hilo(wv, f8)
        wph_, wpl_ = _hilo(wp, f8)
        in_maps.append({
            "xh": x_pairs[b][0], "xl": x_pairs[b][1],
            "wqh": _pairct(wqh_, CT), "wql": _pairct(wql_, CT),
            "wkh": _pairct(wkh_, CT), "wkl": _pairct(wkl_, CT),
            "wvh": _pairct(wvh_, CT), "wvl": _pairct(wvl_, CT),
            "wph": np.ascontiguousarray(wph_.reshape(2, 128, C).transpose(1, 0, 2)),
            "wpl": np.ascontiguousarray(wpl_.reshape(2, 128, C).transpose(1, 0, 2)),
            "dmh": dm_pairs[b][0], "dml": dm_pairs[b][1],
        })
    return in_maps


def kernel(x, distance_matrix, W_qkv, W_proj, b_proj, _results_hook=None):
    from concourse.bass_utils import run_bass_kernel_spmd

    x = np.asarray(x)
    distance_matrix = np.asarray(distance_matrix)
    W_qkv = np.asarray(W_qkv)
    W_proj = np.asarray(W_proj)
    b_proj = np.asarray(b_proj)
    nc = _get_program()
    in_maps = _make_in_maps(x, distance_matrix, W_qkv, W_proj)
    res = run_bass_kernel_spmd(nc, in_maps, list(range(NCORES)))
    if _results_hook is not None:
        _results_hook(res)
    out = np.zeros((B, N, C), dtype=np.float32)
    for core in range(NCORES):
        b = core // HG
        out[b] += res.results[core]["pout"].T.astype(np.float32)
    out += b_proj[None, None, :].astype(np.float32)
    return out


# revision 15
# speedup vs baseline: 1.0088x; 1.0038x over previous
"""Distributed attention kernel for Trainium2 (8 NeuronCores).

Reference computation (B=2, N=2048, C=1024, H=16, D=64, ALPHA=0.5):
    qkv = x @ W_qkv -> q,k,v [B,H,N,D]
    attn = softmax(q @ k^T / sqrt(D))
    attn = 0.5*dm + 0.5*attn
    out  = (attn @ v).reshape(B,N,C) @ W_proj + b_proj

Sharding: 8 cores = 2 batches x 4 head-groups (4 heads each).
Each core computes its head-group's slice end-to-end, including a partial
projection (row-slice of W_proj); host sums the 4 partials per batch.

Speed strategy vs the fp16 baseline: every matmul whose streams tolerate it
runs as an fp8e4m3 DoubleRow matmul (0.5 PE cycles per output column AND 2
contraction rows per partition), with hi+lo error compensation so accuracy
stays at ~fp16 level:
  - qkv projections: x and W split hi/lo fp8 (W pre-scaled x32 so W~N(0,1)
    quantizes in fp8 normal range); 3 accumulation terms hh+hl+lh at 0.75x
    the fp16 PE cost with 256-deep contraction per instruction.
  - dm@v: DoubleRow with slots = m-tile pairs; hi/lo on both dm (host,
    pre-scaled x512) and v (device, unscaled -- a 1/32 downscale would push
    the lo-residual under the fp8 subnormal floor): 3 terms at 0.75x.
  - proj: DoubleRow over the jo row-pair dim, hi/lo on outT and W_proj
    (both x32-scaled); the epilogue writes outT_hi/outT_lo fp8 directly.
  - scores and attn@v stay fp16: any single-fp8 quantization of the q/k or
    exp streams measures ~1.6-3e-2 on the 2e-2 gate (fp8's 3-bit mantissa
    puts ~3.6% rms noise on softmax weights), and hi/lo-compensating a
    moving operand costs exactly the DoubleRow speedup back.
  - exp: ScalarE, scale 2^-13 (undoes the 32x32 weight scales and applies
    D^-0.5) and bias -3.5 folded in; the bias cancels in softmax
    normalization (real q.k score tails reach 8.4 sigma on this input).
  - softmax denominator rides the e@v matmul as a 1/16 ones-column; the
    normalization constant 16/sum(e) lands the lambda=32 output scale that
    keeps outT in fp8-friendly range; pd's 512/32 and pout's 1/1024
    descales fold into existing copies.
"""

import numpy as np

B, N, C, H, D = 2, 2048, 1024, 16, 64
NCORES = 8
HG = 4                # head-groups per batch
HPC = H // HG         # heads per core = 4
DG = HPC * D          # 256: head-group width
SCALE = D ** -0.5

CT = 4                # contraction pair-tiles for qkv (1024 = 4*256)
NQ = N // 512         # 4 q-chunks
MT = N // 128         # 16 m (key) tiles
PT = MT // 2          # 8 m pair-tiles

WS = 32.0             # weight scale (W_qkv, W_proj)
LAM = 32.0            # output scale carried by outT
DMSC = 512.0          # dm host scale; pd = 512*dm@v, descaled 1/32 -> 16
ACT_SCALE = 0.125 / (WS * WS)   # 2^-13
EBIAS = -3.5
ONEC = 1.0 / 16.0     # denominator column: rec2 = 16/sum(e) = LAM*0.5/sum(e)


def _build_program():
    import concourse.bass as bass
    import concourse.bacc as bacc
    import concourse.tile as tile
    from concourse import mybir
    from contextlib import ExitStack

    f32 = mybir.dt.float32
    f16 = mybir.dt.float16
    f8 = mybir.dt.float8e4
    Exp = mybir.ActivationFunctionType.Exp
    DR = mybir.MatmulPerfMode.DoubleRow
    Alu = mybir.AluOpType

    nc = bacc.Bacc()
    xh = nc.declare_dram_parameter("xh", [128, CT, 2, N], f8, isOutput=False)
    xl = nc.declare_dram_parameter("xl", [128, CT, 2, N], f8, isOutput=False)
    wqh = nc.declare_dram_parameter("wqh", [128, CT, 2, DG], f8, isOutput=False)
    wql = nc.declare_dram_parameter("wql", [128, CT, 2, DG], f8, isOutput=False)
    wkh = nc.declare_dram_parameter("wkh", [128, CT, 2, DG], f8, isOutput=False)
    wkl = nc.declare_dram_parameter("wkl", [128, CT, 2, DG], f8, isOutput=False)
    wvh = nc.declare_dram_parameter("wvh", [128, CT, 2, DG], f8, isOutput=False)
    wvl = nc.declare_dram_parameter("wvl", [128, CT, 2, DG], f8, isOutput=False)
    wph = nc.declare_dram_parameter("wph", [128, 2, C], f8, isOutput=False)
    wpl = nc.declare_dram_parameter("wpl", [128, 2, C], f8, isOutput=False)
    dmh = nc.declare_dram_parameter("dmh", [128, PT, 2, N], f8, isOutput=False)
    dml = nc.declare_dram_parameter("dml", [128, PT, 2, N], f8, isOutput=False)
    pout = nc.declare_dram_parameter("pout", [C, N], f16, isOutput=True)

    with tile.TileContext(nc) as tc, ExitStack() as ctx:
        big = ctx.enter_context(tc.tile_pool(name="big", bufs=1))
        epool = ctx.enter_context(tc.tile_pool(name="epool", bufs=6))
        small = ctx.enter_context(tc.tile_pool(name="small", bufs=1))
        outp = ctx.enter_context(tc.tile_pool(name="outp", bufs=4))
        # PSUM: psS 2x[128,1024] (4 banks) + pe0/pe1 (2) + pd (1) + proj (1) = 8
        psS = ctx.enter_context(tc.tile_pool(name="psS", bufs=2, space="PSUM"))
        psE = ctx.enter_context(tc.tile_pool(name="psE", bufs=1, space="PSUM"))
        psD = ctx.enter_context(tc.tile_pool(name="psD", bufs=1, space="PSUM"))

        xh_s = big.tile([128, CT, 2, N], f8)
        xl_s = big.tile([128, CT, 2, N], f8)
        wq_s = {hl: big.tile([128, CT, 2, DG], f8, name=f"wq_s{hl}") for hl in range(2)}
        wk_s = {hl: big.tile([128, CT, 2, DG], f8, name=f"wk_s{hl}") for hl in range(2)}
        wv_s = {hl: big.tile([128, CT, 2, DG], f8, name=f"wv_s{hl}") for hl in range(2)}
        wp_s = {hl: big.tile([128, 2, C], f8, name=f"wp_s{hl}") for hl in range(2)}
        dm_s = {hl: big.tile([128, PT, 2, N], f8, name=f"dm_s{hl}") for hl in range(2)}
        qt = big.tile([128, 2, N], f16)
        kt = big.tile([128, 2, N], f16)
        # e@v stationary: [p, mt, head, D + ones-col], fp16, v unscaled
        vaug = big.tile([128, MT, HPC, D + 1], f16)
        # dm@v stationary: [p, (hi,lo), pt, m-parity, dg], fp8, v unscaled
        vd = big.tile([128, 2, PT, 2, DG], f8)
        oth = big.tile([128, 2, N], f8)
        otl = big.tile([128, 2, N], f8)
        bias_t = big.tile([128, 1], f32)
        ones16 = big.tile([1, D], f16)
        rscratch = nc.dram_tensor("rscratch", [8, 1024], f32)

        nc.vector.memset(bias_t[:, :], EBIAS)
        nc.vector.memset(ones16[:, :], 1.0)
        nc.vector.memset(vaug[:, :, :, D], ONEC)

        # ---- input DMAs: x in (ctpair x n-half) chunks so production can
        # start at ~1.6us and the first-half groups finish by ~7us ----
        for half in range(2):
            nsl = slice(half * 1024, (half + 1) * 1024)
            for cp in range(CT):
                nc.sync.dma_start(out=xh_s[:, cp, :, nsl], in_=xh[:, cp, :, nsl])
                nc.sync.dma_start(out=xl_s[:, cp, :, nsl], in_=xl[:, cp, :, nsl])
                if half == 0 and cp == 0:
                    nc.sync.dma_start(out=wk_s[0][:, :, :, :], in_=wkh[:, :, :, :])
                    nc.sync.dma_start(out=wk_s[1][:, :, :, :], in_=wkl[:, :, :, :])
                    nc.sync.dma_start(out=wq_s[0][:, :, :, :], in_=wqh[:, :, :, :])
                    nc.sync.dma_start(out=wq_s[1][:, :, :, :], in_=wql[:, :, :, :])
        nc.sync.dma_start(out=wv_s[0][:, :, :, :], in_=wvh[:, :, :, :])
        nc.sync.dma_start(out=wv_s[1][:, :, :, :], in_=wvl[:, :, :, :])
        nc.sync.dma_start(out=wp_s[0][:, :, :], in_=wph[:, :, :])
        nc.sync.dma_start(out=wp_s[1][:, :, :], in_=wpl[:, :, :])
        for pt in range(PT):
            nc.sync.dma_start(out=dm_s[0][:, pt, :, :], in_=dmh[:, pt, :, :])
            nc.sync.dma_start(out=dm_s[1][:, pt, :, :], in_=dml[:, pt, :, :])

        # 3 hi/lo accumulation terms: (x_hi,w_hi), (x_hi,w_lo), (x_lo,w_hi)
        def terms(w):
            return ((xh_s, w[0]), (xh_s, w[1]), (xl_s, w[0]))

        # ---- phase 1: q/k production, ct-outer over 6 psum slots per round
        # (round A = n-half 0 groups, round B = n-half 1), overlapping x DMA ----
        slot_tags = ["psS", "psS", "pe0", "pe1", "pd0", "pd1"]

        def qk_round(groups, tags=None):
            ps = {}
            tags = tags or slot_tags
            for i, (w, dst, jo, nq) in enumerate(groups):
                tag = tags[i]
                pool = psS if tag == "psS" else (psE if tag.startswith("pe") else psD)
                ps[i] = pool.tile([128, 512], f32, name=f"g{i}", tag=tag)
            for cp in range(CT):
                for ti in range(3):
                    for i, (w, dst, jo, nq) in enumerate(groups):
                        xs, ws = terms(w)[ti]
                        nc.tensor.matmul(
                            ps[i][:, :],
                            lhsT=ws[:, cp, :, jo * 128:(jo + 1) * 128],
                            rhs=xs[:, cp, :, nq * 512:(nq + 1) * 512],
                            start=(cp == 0 and ti == 0),
                            stop=(cp == CT - 1 and ti == 2),
                            perf_mode=DR,
                        )
            order = sorted(range(len(groups)),
                           key=lambda i: 0 if tags[i] in ("pe0", "pe1") else 1)
            for i in order:
                w, dst, jo, nq = groups[i]
                nc.vector.tensor_copy(dst[:, jo, nq * 512:(nq + 1) * 512], ps[i][:, :])

        qk_round([(wk_s, kt, 0, 0), (wk_s, kt, 0, 1), (wk_s, kt, 1, 0),
                  (wk_s, kt, 1, 1), (wq_s, qt, 0, 0), (wq_s, qt, 0, 1)])

        # ---- v tiles: DR production; epilogue emits vaug fp16 + vd hi/lo fp8 ----
        def v_tile(mt):
            msl = slice(mt * 128, (mt + 1) * 128)
            ps = psE.tile([128, DG], f32, name="ps", tag=f"pe{mt % 2}",
                          padded_shape=[128, 512])
            for cp in range(CT):
                for ti, (xs, ws) in enumerate(terms(wv_s)):
                    nc.tensor.matmul(
                        ps[:, :],
                        lhsT=xs[:, cp, :, msl],
                        rhs=ws[:, cp, :, :],
                        start=(cp == 0 and ti == 0), stop=(cp == CT - 1 and ti == 2),
                        perf_mode=DR,
                    )
            psv = ps[:, :]
            ps4 = bass.AP(tensor=psv.tensor, offset=psv.offset,
                          ap=[list(psv.ap[0]), [D, HPC], [1, D]])
            pt, par = divmod(mt, 2)
            nc.scalar.mul(vaug[:, mt, :, 0:D], ps4, 1.0 / WS)
            hi = vd[:, 0, pt, par, :]
            nc.scalar.mul(hi, psv, 1.0 / WS)
            nc.vector.scalar_tensor_tensor(
                vd[:, 1, pt, par, :], psv, 1.0 / WS, hi,
                op0=Alu.mult, op1=Alu.subtract,
            )

        for mt in range(8):
            v_tile(mt)
        qk_round([(wk_s, kt, 0, 2), (wk_s, kt, 0, 3), (wk_s, kt, 1, 2),
                  (wk_s, kt, 1, 3), (wq_s, qt, 0, 2), (wq_s, qt, 0, 3)])
        for mt in range(8, MT):
            v_tile(mt)
        # final q round avoids psS tags so attention's score psums free up early
        qk_round([(wq_s, qt, 1, 0), (wq_s, qt, 1, 1),
                  (wq_s, qt, 1, 2), (wq_s, qt, 1, 3)], tags=["pd0", "pd1", "pe0", "pe1"])

        # ---- phase 2: attention ----
        def proj_group(nq, co, tag="pd1"):
            qsl = slice(nq * 512, (nq + 1) * 512)
            if tag == "pd1":
                ps = psD.tile([128, 512], f32, name="pj", tag="pd1")
            else:
                ps = psS.tile([128, 512], f32, name="pjS", tag="psS",
                              padded_shape=[128, 1024])
            for ti, (o_s, w_s) in enumerate(((oth, wp_s[0]), (otl, wp_s[0]), (oth, wp_s[1]))):
                nc.tensor.matmul(
                    ps[:, :],
                    lhsT=w_s[:, :, co * 128:(co + 1) * 128],
                    rhs=o_s[:, :, qsl],
                    start=(ti == 0), stop=(ti == 2),
                    perf_mode=DR,
                )
            so = outp.tile([128, 512], f16)
            nc.vector.tensor_scalar_mul(so[:, :], ps[:, :], 1.0 / (WS * LAM))
            nc.sync.dma_start(out=pout[co * 128:(co + 1) * 128, qsl], in_=so[:, :])

        pending_proj = None
        for nq in range(NQ):
            qsl = slice(nq * 512, (nq + 1) * 512)
            for hp in range(2):
                pe0 = psE.tile([D + 1, 512], f32, name="pe0", tag="pe0",
                               padded_shape=[128, 512])
                pe1 = psE.tile([D + 1, 512], f32, name="pe1", tag="pe1",
                               padded_shape=[128, 512])
                pd = psD.tile([128, 512], f32, name="pd", tag="pd0")
                pes = (pe0, pe1)
                for mt in range(MT):
                    msl = slice(mt * 128, (mt + 1) * 128)
                    sps = psS.tile([128, 1024], f32, name="sps", tag="psS")
                    nc.tensor.matmul(
                        sps[:, 0:512],
                        lhsT=kt[0:D, hp, msl], rhs=qt[0:D, hp, qsl],
                        start=True, stop=True,
                    )
                    nc.tensor.matmul(
                        sps[:, 512:1024],
                        lhsT=kt[D:2 * D, hp, msl], rhs=qt[D:2 * D, hp, qsl],
                        start=True, stop=True,
                    )
                    et = epool.tile([128, 1024], f16)
                    nc.scalar.activation(et[:, :], sps[:, :], Exp,
                                         bias=bias_t[:, :], scale=ACT_SCALE)
                    if mt % 2 == 0:
                        pt = mt // 2
                        # dm@v: slots = m-tile pairs; 3 hi/lo terms
                        for ti, (vhl, dhl) in enumerate(((0, 0), (1, 0), (0, 1))):
                            nc.tensor.matmul(
                                pd[:, :],
                                lhsT=vd[:, vhl, pt, :, hp * 128:(hp + 1) * 128],
                                rhs=dm_s[dhl][:, pt, :, qsl],
                                start=(pt == 0 and ti == 0),
                                stop=(pt == PT - 1 and ti == 2),
                                perf_mode=DR,
                            )
                    nc.tensor.matmul(
                        pe0[:, :], lhsT=vaug[:, mt, 2 * hp, :], rhs=et[:, 0:512],
                        start=(mt == 0), stop=(mt == MT - 1),
                    )
                    nc.tensor.matmul(
                        pe1[:, :], lhsT=vaug[:, mt, 2 * hp + 1, :], rhs=et[:, 512:1024],
                        start=(mt == 0), stop=(mt == MT - 1),
                    )
                    if pending_proj is not None and hp == 0 and 1 <= mt <= 8:
                        proj_group(pending_proj, mt - 1)
                # epilogue: normalize softmax part, add dm part, emit outT hi/lo
                slot = nq * 2 + hp
                last = (nq == NQ - 1 and hp == 1)
                if last:
                    pe_s0, pe_s1, pd_s = pe0, pe1, None
                else:
                    pd_s = small.tile([128, 512], f32, name="pd_s", tag="pd_s")
                    nc.vector.tensor_scalar_mul(pd_s[:, :], pd[:, :], 1.0 / LAM)
                    pe_s0 = small.tile([D + 1, 512], f32, name="pe_s0", tag="pe_s0")
                    nc.vector.tensor_copy(pe_s0[:, :], pe0[:, :])
                    pe_s1 = small.tile([D + 1, 512], f32, name="pe_s1", tag="pe_s1")
                    nc.vector.tensor_copy(pe_s1[:, :], pe1[:, :])
                rec2 = small.tile([1, 1024], f16 if last else f32, name="rec2",
                                  tag="rec2l" if last else "rec2")
                for half, ps_ in ((0, pe_s0), (1, pe_s1)):
                    with nc.allow_low_precision(reason="1/r broadcast"):
                        nc.vector.reciprocal(
                            rec2[:, half * 512:(half + 1) * 512], ps_[D:D + 1, :])
                if last:
                    bcp = psS.tile([D, 1024], f32, name="bcp", tag="psS",
                                   padded_shape=[128, 1024])
                    nc.tensor.matmul(bcp[:, 0:512], lhsT=ones16[:, :],
                                     rhs=rec2[:, 0:512], start=True, stop=True)
                    nc.tensor.matmul(bcp[:, 512:1024], lhsT=ones16[:, :],
                                     rhs=rec2[:, 512:1024], start=True, stop=True)
                    bcs = small.tile([D, 1024], f32, name="bcs", tag="bcs")
                    nc.vector.tensor_copy(bcs[:, :], bcp[:, :])
                else:
                    nc.sync.dma_start(out=rscratch[slot:slot + 1, :], in_=rec2[:, :])
                    row = rscratch[slot, :]
                    bc_ap = bass.AP(tensor=row.tensor, offset=row.offset,
                                    ap=[[0, D]] + list(row.ap))
                    bcs = small.tile([D, 1024], f32, name="bcs", tag="bcs")
                    nc.sync.dma_start(out=bcs[:, :], in_=bc_ap)
                for half, ps_ in ((0, pe_s0), (1, pe_s1)):
                    hsl = slice(half * D, (half + 1) * D)
                    t1 = small.tile([128, 512], f32, name="t1", tag="t1")
                    nc.vector.tensor_mul(
                        t1[hsl, :], ps_[0:D, :], bcs[:, half * 512:(half + 1) * 512])
                    t2 = small.tile([128, 512], f16, name="t2", tag="t2")
                    if last:
                        nc.vector.scalar_tensor_tensor(
                            t2[hsl, :], pd[hsl, :], 1.0 / LAM, t1[hsl, :],
                            op0=Alu.mult, op1=Alu.add,
                        )
                    else:
                        nc.vector.tensor_add(t2[hsl, :], t1[hsl, :], pd_s[hsl, :])
                    hi = oth[hsl, hp, qsl]
                    nc.vector.tensor_copy(hi, t2[hsl, :])
                    nc.vector.scalar_tensor_tensor(
                        otl[hsl, hp, qsl], t2[hsl, :], 1.0, hi,
                        op0=Alu.mult, op1=Alu.subtract,
                    )
            pending_proj = nq
        for co in range(C // 128):
            proj_group(NQ - 1, co, tag="pd1" if co % 2 == 0 else "psS")
    nc.compile()
    return nc


_PROGRAM = None


def _get_program():
    global _PROGRAM
    if _PROGRAM is None:
        _PROGRAM = _build_program()
    return _PROGRAM


def _hilo(a, f8):
    hi = np.asarray(a, dtype=f8)
    lo = np.asarray(a - hi.astype(np.float32), dtype=f8)
    return hi, lo


def _pairct(a, nt):
    """[K, F] -> [128, nt, 2, F] with (p, t, i) <-> row t*256 + i*128 + p."""
    K, F = a.shape
    assert K == nt * 256
    return np.ascontiguousarray(a.reshape(nt, 2, 128, F).transpose(2, 0, 1, 3))


def _make_in_maps(x, distance_matrix, W_qkv, W_proj):
    import ml_dtypes
    f8 = ml_dtypes.float8_e4m3

    in_maps = []
    xTs = [np.ascontiguousarray(x[b].T).astype(np.float32) for b in range(B)]
    dmSs = [np.ascontiguousarray(DMSC * distance_matrix[b, 0].T).astype(np.float32)
            for b in range(B)]
    dm_pairs = []
    for b in range(B):
        dh, dl = _hilo(dmSs[b], f8)
        dm_pairs.append((_pairct(dh, PT), _pairct(dl, PT)))
    x_pairs = []
    for b in range(B):
        xhi, xlo = _hilo(xTs[b], f8)
        x_pairs.append((_pairct(xhi, CT), _pairct(xlo, CT)))

    for core in range(NCORES):
        b, hg = divmod(core, HG)
        sl = slice(hg * DG, (hg + 1) * DG)
        wq = WS * W_qkv[:, sl].astype(np.float32)
        wk = WS * W_qkv[:, C + hg * DG:C + (hg + 1) * DG].astype(np.float32)
        wv = WS * W_qkv[:, 2 * C + hg * DG:2 * C + (hg + 1) * DG].astype(np.float32)
        wp = WS * W_proj[sl, :].astype(np.float32)
        wqh_, wql_ = _hilo(wq, f8)
        wkh_, wkl_ = _hilo(wk, f8)
        wvh_, wvl_ = _hilo(wv, f8)
        wph_, wpl_ = _hilo(wp, f8)
        in_maps.append({
            "xh": x_pairs[b][0], "xl": x_pairs[b][1],
            "wqh": _pairct(wqh_, CT), "wql": _pairct(wql_, CT),
            "wkh": _pairct(wkh_, CT), "wkl": _pairct(wkl_, CT),
            "wvh": _pairct(wvh_, CT), "wvl": _pairct(wvl_, CT),
            "wph": np.ascontiguousarray(wph_.reshape(2, 128, C).transpose(1, 0, 2)),
            "wpl": np.ascontiguousarray(wpl_.reshape(2, 128, C).transpose(1, 0, 2)),
            "dmh": dm_pairs[b][0], "dml": dm_pairs[b][1],
        })
    return in_maps


def kernel(x, distance_matrix, W_qkv, W_proj, b_proj, _results_hook=None):
    from concourse.bass_utils import run_bass_kernel_spmd

    x = np.asarray(x)
    distance_matrix = np.asarray(distance_matrix)
    W_qkv = np.asarray(W_qkv)
    W_proj = np.asarray(W_proj)
    b_proj = np.asarray(b_proj)
    nc = _get_program()
    in_maps = _make_in_maps(x, distance_matrix, W_qkv, W_proj)
    res = run_bass_kernel_spmd(nc, in_maps, list(range(NCORES)))
    if _results_hook is not None:
        _results_hook(res)
    out = np.zeros((B, N, C), dtype=np.float32)
    for core in range(NCORES):
        b = core // HG
        out[b] += res.results[core]["pout"].T.astype(np.float32)
    out += b_proj[None, None, :].astype(np.float32)
    return out
